# revision 5
# baseline (speedup 1.0000x reference)
"""LitEma shadow-param EMA update on 8 TRN2 NeuronCores.

new_shadow = shadow - (1 - decay_val) * (shadow - params)
decay_val  = min(0.9999, (1 + nu) / (10 + nu)),  nu = num_updates + 1

Fully data-parallel elementwise op: the flat 64M-element arrays are split
evenly across the 8 cores; no cross-core communication.
"""

import numpy as np

import concourse.bass as bass
import concourse.bacc as bacc
import concourse.tile as tile
from concourse import mybir
from concourse.bass_utils import run_bass_kernel_spmd

N_TOTAL = 67108864
N_CORES = 8
N_PER_CORE = N_TOTAL // N_CORES  # 8388608
P = 128          # SBUF partitions
M = 8192         # free-dim elements per tile (32 KB/partition fp32)
NTILES = N_PER_CORE // (P * M)   # 8 tiles per core, 4 MiB per DMA
DECAY = 0.9999


def _build_nc(
    neg_omd: float, n_per_core: int = N_PER_CORE, m: int = M, reps: int = 1
) -> bass.Bass:
    """Per-core program: out = shadow + neg_omd * (shadow - params).

    reps > 1 unrolls the whole pass multiple times (same inputs/outputs) so
    one NEFF execution measures reps kernel iterations back-to-back.
    """
    ntiles = n_per_core // (P * m)
    assert ntiles * P * m == n_per_core
    nc = bacc.Bacc(trn_type="TRN2", target_bir_lowering=False, debug=False)
    shadow = nc.declare_dram_parameter(
        "shadow", [n_per_core], mybir.dt.float32, isOutput=False
    )
    params = nc.declare_dram_parameter(
        "params", [n_per_core], mybir.dt.float32, isOutput=False
    )
    out = nc.declare_dram_parameter(
        "out", [n_per_core], mybir.dt.float32, isOutput=True
    )
    sh = shadow.ap().rearrange("(n p m) -> n p m", p=P, m=m)
    pr = params.ap().rearrange("(n p m) -> n p m", p=P, m=m)
    ot = out.ap().rearrange("(n p m) -> n p m", p=P, m=m)

    with tile.TileContext(nc) as tc:
        with (
            tc.tile_pool(name="s", bufs=3) as sp,
            tc.tile_pool(name="p", bufs=3) as pp,
        ):
            for _ in range(reps):
                for i in range(ntiles):
                    s = sp.tile([P, m], mybir.dt.float32)
                    p = pp.tile([P, m], mybir.dt.float32)
                    nc.sync.dma_start(s[:], sh[i])
                    nc.sync.dma_start(p[:], pr[i])
                    # p = shadow - params
                    nc.vector.tensor_sub(p[:], s[:], p[:])
                    # s = (p * -omd) + s  ==  shadow - omd*(shadow - params)
                    nc.vector.scalar_tensor_tensor(
                        s[:], p[:], neg_omd, s[:],
                        mybir.AluOpType.mult, mybir.AluOpType.add,
                    )
                    nc.sync.dma_start(ot[i], s[:])
    nc.compile()
    return nc


_NC_CACHE: dict[float, bass.Bass] = {}


def _get_nc(neg_omd: float) -> bass.Bass:
    nc = _NC_CACHE.get(neg_omd)
    if nc is None:
        nc = _build_nc(neg_omd)
        _NC_CACHE[neg_omd] = nc
    return nc


def _one_minus_decay(num_updates) -> float:
    nu = float(int(num_updates) + 1)
    decay_val = min(DECAY, (1.0 + nu) / (10.0 + nu))
    return 1.0 - decay_val


def shard_inputs(shadow, params):
    shadow = np.ascontiguousarray(np.asarray(shadow, dtype=np.float32)).reshape(-1)
    params = np.ascontiguousarray(np.asarray(params, dtype=np.float32)).reshape(-1)
    assert shadow.size == N_TOTAL and params.size == N_TOTAL
    return [
        {
            "shadow": shadow[i * N_PER_CORE : (i + 1) * N_PER_CORE],
            "params": params[i * N_PER_CORE : (i + 1) * N_PER_CORE],
        }
        for i in range(N_CORES)
    ]


def kernel(shadow, params, num_updates):
    neg_omd = -_one_minus_decay(num_updates)
    nc = _get_nc(neg_omd)
    in_maps = shard_inputs(shadow, params)
    res = run_bass_kernel_spmd(nc, in_maps, list(range(N_CORES)))
    return np.concatenate([res.results[i]["out"].reshape(-1) for i in range(N_CORES)])
